# revision 2
# baseline (speedup 1.0000x reference)
"""Trainium2 Bass kernel for nn_Expand_36610301231376.

kernel(**inputs) takes the FULL unsharded inputs (as in reference.setup_inputs)
and returns the FULL (16, 512, 56, 56) float32 output.

Strategy: pure data parallel over batch B=16 across 8 NeuronCores (2 batches
per core); all parameters replicated. Inside each core, tokens (56x56=3136)
are processed channel-major in 7 chunks of 448 (8 image rows); row-local
attention runs on 2-row blocks of 112 tokens. All matmuls are bf16 with fp32
PSUM accumulation; LayerNorm statistics are computed with ones-matmuls on the
TensorEngine and rsqrt on (1,448) stat rows, then broadcast back via a PE
rank-1 matmul. The conv bias b_conv1 cancels exactly in LayerNorm and the
1/sqrt(512) attention scale, LN gammas/betas, positional encodings and
projection biases are folded into host-precomputed constants.

The two batches of each chunk are software-pipelined (phase-interleaved
emission) so the TensorEngine stays busy through the elementwise phases.
"""
import sys

if "/opt/trn_rl_repo" not in sys.path:
    sys.path.insert(0, "/opt/trn_rl_repo")

import numpy as np
import orjson

# ----------------------------------------------------------------------------
# BIR post-pass: this container's walrus build supports only ONE sync-wait per
# instruction; split multi-wait instructions into single-wait NoOps.
# ----------------------------------------------------------------------------
_wcounter = [0]


def _split_block(instructions):
    out, changed = [], False
    for inst in instructions:
        si = inst.get("sync_info")
        waits = (si or {}).get("on_wait") or []
        if len(waits) > 1:
            changed = True
            for w in waits[:-1]:
                _wcounter[0] += 1
                nop = {
                    "engine": inst["engine"], "ins": [], "outs": [],
                    "name": f"I-wsplit-{_wcounter[0]}", "opcode": "NoOp",
                    "sync_info": {"on_update": [], "on_wait": [w]},
                }
                if "debug" in inst:
                    nop["debug"] = inst["debug"]
                out.append(nop)
            si["on_wait"] = [waits[-1]]
        out.append(inst)
    return out, changed


def _split_multi_waits_json(bir_json: bytes) -> bytes:
    m = orjson.loads(bir_json)
    changed = False
    for fn in m.get("functions", []):
        for blk in fn.get("blocks", []):
            insts = blk.get("instructions")
            if insts:
                blk["instructions"], ch = _split_block(insts)
                changed = changed or ch
    return orjson.dumps(m) if changed else bir_json


def _install_patch():
    import concourse.bass as bass

    if getattr(bass.Bass, "_wait_split_installed", False):
        return
    orig = bass.Bass.to_json_bytes

    def to_json_bytes(self):
        return _split_multi_waits_json(orig(self))

    bass.Bass.to_json_bytes = to_json_bytes
    bass.Bass._wait_split_installed = True


# ----------------------------------------------------------------------------
# Problem constants (hardcoded from the problem spec)
# ----------------------------------------------------------------------------
B = 16
N_CORES = 8
B_LOC = B // N_CORES
T_LEN, T_DIM = 149, 768
H = W = 56
S_DIM = 512
N_TOK = H * W           # 3136
CH = 448                # tokens per chunk (8 image rows)
NCHUNK = N_TOK // CH    # 7
NBLK = CH // 112        # 4 two-row attention blocks per chunk
EPS = 1e-5


# ----------------------------------------------------------------------------
# Device program
# ----------------------------------------------------------------------------
def _build_program():
    import concourse.bass as bass
    import concourse.tile as tile
    from concourse import mybir

    F32 = mybir.dt.float32
    BF16 = mybir.dt.bfloat16
    AF = mybir.ActivationFunctionType
    OP = mybir.AluOpType

    nc = bass.Bass(trn_type="TRN2", target_bir_lowering=False, debug=False)
    din = {}
    for name, shape, dt_ in [
        ("x0", (128, B_LOC, T_DIM), BF16), ("x1", (32, B_LOC, T_DIM), BF16),
        ("xT", (128, 6, B_LOC, T_LEN), BF16),
        ("w1t", (128, 2, N_TOK), BF16),
        ("wqgt", (128, 6, S_DIM), BF16), ("uq", (1, S_DIM), BF16),
        ("wkt", (128, 4, S_DIM), BF16),
        ("ones", (128, 128), BF16), ("ident", (128, 128), BF16),
        ("cq", (128, 4, N_TOK), F32), ("bks", (128, 4), F32),
        ("g2", (128, 4), F32), ("pe2p", (128, 4, N_TOK), F32),
        ("masks", (112, 112), F32),
        ("y", (B_LOC, 128, 4, N_TOK), F32),
    ]:
        din[name] = nc.dram_tensor(name, list(shape), dt_, kind="ExternalInput").ap()
    dout = nc.dram_tensor("out", [B_LOC, 128, 4, N_TOK], F32,
                          kind="ExternalOutput").ap()

    from contextlib import ExitStack

    with nc.allow_low_precision(reason="bf16 matmul operands, fp32 accumulate"), \
         tile.TileContext(nc) as tc, ExitStack() as ctx:
        singles = ctx.enter_context(tc.tile_pool(name="singles", bufs=1))
        io3 = ctx.enter_context(tc.tile_pool(name="io3", bufs=3))
        io2 = ctx.enter_context(tc.tile_pool(name="io2", bufs=2))
        wk2 = ctx.enter_context(tc.tile_pool(name="wk2", bufs=2))
        att = ctx.enter_context(tc.tile_pool(name="att", bufs=3))
        sc = ctx.enter_context(tc.tile_pool(name="sc", bufs=2))
        eqp = ctx.enter_context(tc.tile_pool(name="eqp", bufs=1))
        ps_mm = ctx.enter_context(tc.tile_pool(name="ps_mm", bufs=4, space="PSUM"))
        ps_att = ctx.enter_context(tc.tile_pool(name="ps_att", bufs=2, space="PSUM"))
        ps_st = ctx.enter_context(tc.tile_pool(name="ps_st", bufs=2, space="PSUM"))

        def load(name, shape, dt_):
            t = singles.tile(list(shape), dt_, tag=name)
            nc.sync.dma_start(out=t, in_=din[name])
            return t

        x0 = load("x0", (128, B_LOC, T_DIM), BF16)
        x1 = load("x1", (32, B_LOC, T_DIM), BF16)
        xT = load("xT", (128, 6, B_LOC, T_LEN), BF16)
        w1t = load("w1t", (128, 2, N_TOK), BF16)
        wqgt = load("wqgt", (128, 6, S_DIM), BF16)
        uq = load("uq", (1, S_DIM), BF16)
        wkt = load("wkt", (128, 4, S_DIM), BF16)
        ones = load("ones", (128, 128), BF16)
        ident = load("ident", (128, 128), BF16)
        bks = load("bks", (128, 4), F32)
        g2 = load("g2", (128, 4), F32)
        masks = load("masks", (112, 112), F32)
        ones_col = ones[:, 0:1]
        ones_row = ones[0:1, :]
        eps1 = singles.tile([1, 1], F32)
        nc.vector.memset(eps1, EPS)

        x_k = [(x0, 128), (x1, 21)]

        def phase_eq(st):
            # E_q^T[l, o] = sum_c x[b][l, c] * Wqg[o, c]  (149 x 512, 2 l-chunks)
            b = st["b"]
            eq = []
            for lc, (off, kv) in enumerate([(0, 128), (128, 21)]):
                pe_ = ps_mm.tile([128, S_DIM], F32, tag="mm")
                for kc in range(6):
                    nc.tensor.matmul(pe_[:kv, :], xT[:, kc, b, off:off + kv],
                                     wqgt[:, kc, :], start=(kc == 0),
                                     stop=(kc == 5))
                et = eqp.tile([128, S_DIM], BF16, tag=f"eq{b}_{lc}")
                nc.vector.tensor_copy(out=et[:kv, :], in_=pe_[:kv, :])
                eq.append(et)
            st["eq"] = eq

        def phase_load(st):
            b, cols = st["b"], st["cols"]
            y_t = io3.tile([128, 4, CH], F32, tag="y")
            nc.sync.dma_start(out=y_t, in_=din["y"][b, :, :, cols])
            ybf = wk2.tile([128, 4, CH], BF16, tag="ybf")
            nc.scalar.activation(out=ybf, in_=y_t, func=AF.Copy)
            st["y_t"], st["ybf"] = y_t, ybf

        def phase_xe(st):
            b, cols = st["b"], st["cols"]
            xe = wk2.tile([128, 6, CH], BF16, tag="xe")
            sq = wk2.tile([128, 6, CH], BF16, tag="sq")
            for m in range(6):
                pxe = ps_mm.tile([128, CH], F32, tag="mm")
                for ik, (xt, kv) in enumerate(x_k):
                    nc.tensor.matmul(
                        pxe, xt[:kv, b, m * 128:(m + 1) * 128],
                        w1t[:kv, ik, cols], start=(ik == 0), stop=(ik == 1))
                if m % 2 == 0:
                    nc.vector.tensor_copy(out=xe[:, m, :], in_=pxe)
                else:
                    nc.scalar.activation(out=xe[:, m, :], in_=pxe, func=AF.Copy)
                nc.scalar.square(out=sq[:, m, :], in_=pxe)
            ps1 = ps_st.tile([1, CH], F32, tag="st")
            for m in range(6):
                nc.tensor.matmul(ps1, ones_col, xe[:, m, :],
                                 start=(m == 0), stop=(m == 5))
            pq1 = ps_st.tile([1, CH], F32, tag="st")
            for m in range(6):
                nc.tensor.matmul(pq1, ones_col, sq[:, m, :],
                                 start=(m == 0), stop=(m == 5))
            st["xe"], st["ps1"], st["pq1"] = xe, ps1, pq1

        def _rsqrt_row(psum_s, psum_q, inv_d):
            mrow = sc.tile([1, CH], BF16, tag="ma")
            nc.scalar.activation(out=mrow, in_=psum_s, func=AF.Copy, scale=-inv_d)
            vrow = sc.tile([1, CH], F32, tag="vb")
            nc.vector.tensor_scalar_mul(out=vrow, in0=psum_q, scalar1=inv_d)
            t = sc.tile([1, CH], F32, tag="t")
            nc.vector.tensor_mul(out=t, in0=mrow, in1=mrow)
            nc.vector.tensor_tensor(out=vrow, in0=vrow, in1=t, op=OP.subtract)
            nc.scalar.activation(out=vrow, in_=vrow, func=AF.Sqrt, bias=eps1)
            rrow = sc.tile([1, CH], BF16, tag="rr")
            nc.vector.reciprocal(out=rrow, in_=vrow)
            return mrow, rrow

        def phase_stats1(st):
            mrow1, rrow1 = _rsqrt_row(st["ps1"], st["pq1"], 1.0 / T_DIM)
            prb = ps_st.tile([128, CH], F32, tag="st")
            nc.tensor.matmul(prb, ones_row, rrow1, start=True, stop=True)
            r1b = wk2.tile([128, CH], F32, tag="r1b")
            nc.vector.tensor_copy(out=r1b, in_=prb)
            st["mrow1"], st["r1b"] = mrow1, r1b

        def phase_q(st):
            cols, mrow1, r1b = st["cols"], st["mrow1"], st["r1b"]
            cq_t, eq = st["cq_t"], st["eq"]
            q = wk2.tile([128, 4, CH], BF16, tag="q")
            for oc in range(4):
                pq = ps_mm.tile([128, CH], F32, tag="mm")
                for lc, kv in [(0, 128), (1, 21)]:
                    nc.tensor.matmul(
                        pq, eq[lc][:kv, oc * 128:(oc + 1) * 128],
                        w1t[:kv, lc, cols], start=(lc == 0), stop=False)
                nc.tensor.matmul(pq, uq[:, oc * 128:(oc + 1) * 128], mrow1,
                                 start=False, stop=True)
                nc.vector.tensor_mul(out=q[:, oc, :], in0=pq, in1=r1b)
                nc.vector.tensor_add(out=q[:, oc, :], in0=q[:, oc, :],
                                     in1=cq_t[:, oc, :])
            st["q"] = q

        def phase_stats2a(st):
            ybf = st["ybf"]
            sq2 = wk2.tile([128, 6, CH], BF16, tag="sq")
            nc.scalar.square(out=sq2[:, 0:4, :], in_=ybf)
            ps2 = ps_st.tile([1, CH], F32, tag="st")
            for m in range(4):
                nc.tensor.matmul(ps2, ones_col, ybf[:, m, :],
                                 start=(m == 0), stop=(m == 3))
            pq2 = ps_st.tile([1, CH], F32, tag="st")
            for m in range(4):
                nc.tensor.matmul(pq2, ones_col, sq2[:, m, :],
                                 start=(m == 0), stop=(m == 3))
            st["ps2"], st["pq2"] = ps2, pq2

        def phase_stats2b(st):
            mrow2, rrow2 = _rsqrt_row(st["ps2"], st["pq2"], 1.0 / S_DIM)
            pmb2 = ps_st.tile([128, CH], F32, tag="st")
            nc.tensor.matmul(pmb2, ones_row, mrow2, start=True, stop=True)
            prb2 = ps_st.tile([128, CH], F32, tag="st")
            nc.tensor.matmul(prb2, ones_row, rrow2, start=True, stop=True)
            r2b = wk2.tile([128, CH], F32, tag="r2b")
            nc.vector.tensor_copy(out=r2b, in_=prb2)
            c2b = wk2.tile([128, CH], F32, tag="c2b")
            nc.vector.tensor_tensor(out=c2b, in0=pmb2, in1=r2b, op=OP.mult)
            st["r2b"], st["c2b"] = r2b, c2b

        def phase_ny(st):
            y_t, r2b, c2b, pe2_t = st["y_t"], st["r2b"], st["c2b"], st["pe2_t"]
            ny = wk2.tile([128, 4, CH], BF16, tag="ny")
            nyf = wk2.tile([128, 4, CH], F32, tag="nyf")
            for co in range(4):
                nc.vector.tensor_mul(out=nyf[:, co, :], in0=y_t[:, co, :], in1=r2b)
                nc.gpsimd.tensor_add(out=nyf[:, co, :], in0=nyf[:, co, :], in1=c2b)
                nc.scalar.activation(out=nyf[:, co, :], in_=nyf[:, co, :],
                                     func=AF.Identity, scale=g2[:, co:co + 1])
                nc.vector.tensor_tensor(out=ny[:, co, :], in0=nyf[:, co, :],
                                        in1=pe2_t[:, co, :], op=OP.add)
            st["ny"] = ny

        def phase_kv(st):
            ny = st["ny"]
            k = wk2.tile([128, 4, CH], BF16, tag="k")
            for oc in range(4):
                pk = ps_mm.tile([128, CH], F32, tag="mm")
                for kc in range(4):
                    nc.tensor.matmul(
                        pk, wkt[:, kc, oc * 128:(oc + 1) * 128],
                        ny[:, kc, :], start=(kc == 0), stop=(kc == 3))
                nc.vector.tensor_scalar(out=k[:, oc, :], in0=pk,
                                        scalar1=bks[:, oc:oc + 1],
                                        scalar2=None, op0=OP.add)
            v = wk2.tile([112, 4, S_DIM], BF16, tag="v")
            for blk in range(NBLK):
                tb = slice(blk * 112, (blk + 1) * 112)
                for co in range(4):
                    pt = ps_att.tile([112, 128], BF16, tag="at")
                    nc.tensor.transpose(pt, ny[:, co, tb], ident)
                    if co % 2 == 0:
                        nc.vector.tensor_copy(
                            out=v[:, blk, co * 128:(co + 1) * 128], in_=pt)
                    else:
                        nc.scalar.activation(
                            out=v[:, blk, co * 128:(co + 1) * 128], in_=pt,
                            func=AF.Copy)
            st["k"], st["v"] = k, v

        def phase_att(st):
            b, cols = st["b"], st["cols"]
            q, k, v, y_t = st["q"], st["k"], st["v"], st["y_t"]
            out_t = io2.tile([128, 4, CH], F32, tag="out")
            for blk in range(NBLK):
                tb = slice(blk * 112, (blk + 1) * 112)
                psc = ps_att.tile([112, 112], F32, tag="at")
                for oc in range(4):
                    nc.tensor.matmul(psc, q[:, oc, tb], k[:, oc, tb],
                                     start=(oc == 0), stop=(oc == 3))
                e_t = att.tile([112, 112], F32, tag="e")
                nc.vector.tensor_add(out=e_t, in0=psc, in1=masks)
                den = att.tile([112, 1], F32, tag="den")
                nc.scalar.activation(out=e_t, in_=e_t, func=AF.Exp, accum_out=den)
                nc.vector.reciprocal(out=den, in_=den)
                attn = att.tile([112, 112], BF16, tag="attn")
                nc.vector.tensor_scalar_mul(out=attn, in0=e_t, scalar1=den)
                pat = ps_att.tile([112, 112], BF16, tag="at")
                nc.tensor.transpose(pat, attn, ident[:112, :112])
                attnT = att.tile([112, 112], BF16, tag="attnT")
                nc.vector.tensor_copy(out=attnT, in_=pat)
                pav = ps_mm.tile([128, 4, 112], F32, tag="mm")
                for co in range(4):
                    nc.tensor.matmul(pav[:, co, :],
                                     v[:, blk, co * 128:(co + 1) * 128],
                                     attnT, start=True, stop=True)
                nc.vector.tensor_add(out=out_t[:, :, tb], in0=pav,
                                     in1=y_t[:, :, tb])
            nc.sync.dma_start(out=dout[b, :, :, cols], in_=out_t)

        eq_state = {}
        for ich in range(NCHUNK):
            cols = slice(ich * CH, (ich + 1) * CH)
            cq_t = io2.tile([128, 4, CH], F32, tag="cq")
            nc.sync.dma_start(out=cq_t, in_=din["cq"][:, :, cols])
            pe2_t = io2.tile([128, 4, CH], F32, tag="pe2")
            nc.sync.dma_start(out=pe2_t, in_=din["pe2p"][:, :, cols])

            s0 = {"b": 0, "cols": cols, "cq_t": cq_t, "pe2_t": pe2_t}
            s1 = {"b": 1, "cols": cols, "cq_t": cq_t, "pe2_t": pe2_t}
            if ich == 0:
                phase_eq(s0)
                phase_eq(s1)
                eq_state[0], eq_state[1] = s0["eq"], s1["eq"]
            else:
                s0["eq"], s1["eq"] = eq_state[0], eq_state[1]

            phase_load(s0)
            phase_xe(s0)
            phase_load(s1)
            phase_stats1(s0)
            phase_stats2a(s0)
            phase_xe(s1)
            phase_stats2b(s0)
            phase_q(s0)
            phase_stats1(s1)
            phase_ny(s0)
            phase_stats2a(s1)
            phase_stats2b(s1)
            phase_q(s1)
            phase_kv(s0)
            phase_ny(s1)
            phase_att(s0)
            phase_kv(s1)
            phase_att(s1)
    return nc


# ----------------------------------------------------------------------------
# Host-side preparation
# ----------------------------------------------------------------------------
def _make_const_inputs(W_conv1, b_conv1, ln1_g, ln1_b, ln2_g, ln2_b,
                       pe_wave, pe_spec, Wq, bq, Wk, bk):
    import ml_dtypes
    f = np.float32
    bf = ml_dtypes.bfloat16
    s = np.float32(S_DIM) ** np.float32(-0.25)

    w1t = np.zeros((128, 2, N_TOK), dtype=f)
    w1T = W_conv1.T.astype(f)
    w1t[:, 0, :] = w1T[:128]
    w1t[:21, 1, :] = w1T[128:]

    wqg = (Wq * ln1_g[None, :]).astype(f) * s
    wqgt = wqg.T.reshape(6, 128, S_DIM).transpose(1, 0, 2).copy()
    uq = (Wq @ ln1_g).astype(f)[None, :] * s

    pe_w = pe_wave.reshape(T_DIM, N_TOK).astype(f)
    cq = (Wq @ (ln1_b[:, None] + pe_w)).astype(f) * s + (bq[:, None] * s).astype(f)
    cq = cq.reshape(4, 128, N_TOK).transpose(1, 0, 2).copy()

    wkt = (Wk.T * s).astype(f).reshape(4, 128, S_DIM).transpose(1, 0, 2).copy()
    bks = (bk * s).astype(f).reshape(4, 128).T.copy()
    g2 = ln2_g.astype(f).reshape(4, 128).T.copy()

    pe2p = (pe_spec.reshape(S_DIM, N_TOK) + ln2_b[:, None]).astype(f)
    pe2p = pe2p.reshape(4, 128, N_TOK).transpose(1, 0, 2).copy()

    masks = np.full((112, 112), -1e30, dtype=f)
    for sb in range(2):
        masks[sb * 56:(sb + 1) * 56, sb * 56:(sb + 1) * 56] = 0.0

    return {
        "w1t": w1t.astype(bf), "wqgt": wqgt.astype(bf), "uq": uq.astype(bf),
        "cq": cq, "wkt": wkt.astype(bf), "bks": bks, "g2": g2,
        "pe2p": pe2p, "masks": masks,
        "ones": np.ones((128, 128), dtype=bf),
        "ident": np.eye(128, dtype=bf),
    }


def _make_core_inputs(consts, x_shard, y_shard):
    import ml_dtypes
    f = np.float32
    bf = ml_dtypes.bfloat16
    x0 = x_shard[:, :128, :].transpose(1, 0, 2).astype(bf).copy()
    x1 = np.zeros((32, B_LOC, T_DIM), dtype=bf)
    x1[:21] = x_shard[:, 128:, :].transpose(1, 0, 2).astype(bf)
    # xT[ci, kc, b, l] = x[b, l, kc*128+ci]
    xT = x_shard.transpose(2, 0, 1).reshape(6, 128, B_LOC, T_LEN)\
        .transpose(1, 0, 2, 3).astype(bf).copy()
    y = y_shard.reshape(B_LOC, 4, 128, N_TOK).transpose(0, 2, 1, 3).astype(f).copy()
    m = {"x0": x0, "x1": x1, "xT": xT, "y": y}
    m.update(consts)
    return m


_cached_nc = [None]


def kernel(x, y, W_conv1, b_conv1, ln1_g, ln1_b, ln2_g, ln2_b,
           pe_wave, pe_spec, Wq, bq, Wk, bk):
    _install_patch()
    from concourse.bass_utils import run_bass_kernel_spmd

    x = np.asarray(x, dtype=np.float32)
    y = np.asarray(y, dtype=np.float32)
    consts = _make_const_inputs(
        np.asarray(W_conv1, np.float32), np.asarray(b_conv1, np.float32),
        np.asarray(ln1_g, np.float32), np.asarray(ln1_b, np.float32),
        np.asarray(ln2_g, np.float32), np.asarray(ln2_b, np.float32),
        np.asarray(pe_wave, np.float32), np.asarray(pe_spec, np.float32),
        np.asarray(Wq, np.float32), np.asarray(bq, np.float32),
        np.asarray(Wk, np.float32), np.asarray(bk, np.float32))
    in_maps = [
        _make_core_inputs(consts, x[B_LOC * i:B_LOC * (i + 1)],
                          y[B_LOC * i:B_LOC * (i + 1)])
        for i in range(N_CORES)
    ]

    if _cached_nc[0] is None:
        _cached_nc[0] = _build_program()
    nc = _cached_nc[0]

    res = run_bass_kernel_spmd(nc, in_maps, core_ids=list(range(N_CORES)))
    outs = []
    for i in range(N_CORES):
        o = res.results[i]["out"]  # (B_LOC, 128, 4, N_TOK)
        outs.append(o.transpose(0, 2, 1, 3).reshape(B_LOC, S_DIM, H, W))
    return np.concatenate(outs, axis=0).astype(np.float32)


# revision 3
# speedup vs baseline: 1.1934x; 1.1934x over previous
"""Trainium2 Bass kernel for nn_Expand_36610301231376.

kernel(**inputs) takes the FULL unsharded inputs (as in reference.setup_inputs)
and returns the FULL (16, 512, 56, 56) float32 output.

Strategy: pure data parallel over batch B=16 across 8 NeuronCores (2 batches
per core); all parameters replicated. Inside each core, tokens (56x56=3136)
are processed channel-major in 7 chunks of 448 (8 image rows); row-local
attention runs on 2-row blocks of 112 tokens. All matmuls are bf16 with fp32
PSUM accumulation; LayerNorm statistics are computed with ones-matmuls on the
TensorEngine and rsqrt on (1,448) stat rows, then broadcast back via a PE
rank-1 matmul. The conv bias b_conv1 cancels exactly in LayerNorm and the
1/sqrt(512) attention scale, LN gammas/betas, positional encodings and
projection biases are folded into host-precomputed constants.

The two batches of each chunk are software-pipelined (phase-interleaved
emission) so the TensorEngine stays busy through the elementwise phases.
"""
import sys

if "/opt/trn_rl_repo" not in sys.path:
    sys.path.insert(0, "/opt/trn_rl_repo")

import numpy as np
import orjson

# ----------------------------------------------------------------------------
# BIR post-pass: this container's walrus build supports only ONE sync-wait per
# instruction; split multi-wait instructions into single-wait NoOps.
# ----------------------------------------------------------------------------
_wcounter = [0]


def _split_block(instructions):
    out, changed = [], False
    for inst in instructions:
        si = inst.get("sync_info")
        waits = (si or {}).get("on_wait") or []
        if len(waits) > 1:
            changed = True
            for w in waits[:-1]:
                _wcounter[0] += 1
                nop = {
                    "engine": inst["engine"], "ins": [], "outs": [],
                    "name": f"I-wsplit-{_wcounter[0]}", "opcode": "NoOp",
                    "sync_info": {"on_update": [], "on_wait": [w]},
                }
                if "debug" in inst:
                    nop["debug"] = inst["debug"]
                out.append(nop)
            si["on_wait"] = [waits[-1]]
        out.append(inst)
    return out, changed


def _split_multi_waits_json(bir_json: bytes) -> bytes:
    m = orjson.loads(bir_json)
    changed = False
    for fn in m.get("functions", []):
        for blk in fn.get("blocks", []):
            insts = blk.get("instructions")
            if insts:
                blk["instructions"], ch = _split_block(insts)
                changed = changed or ch
    return orjson.dumps(m) if changed else bir_json


def _install_patch():
    import concourse.bass as bass

    if getattr(bass.Bass, "_wait_split_installed", False):
        return
    orig = bass.Bass.to_json_bytes

    def to_json_bytes(self):
        return _split_multi_waits_json(orig(self))

    bass.Bass.to_json_bytes = to_json_bytes
    bass.Bass._wait_split_installed = True


# ----------------------------------------------------------------------------
# Problem constants (hardcoded from the problem spec)
# ----------------------------------------------------------------------------
B = 16
N_CORES = 8
B_LOC = B // N_CORES
T_LEN, T_DIM = 149, 768
H = W = 56
S_DIM = 512
N_TOK = H * W           # 3136
CH = 448                # tokens per chunk (8 image rows)
NCHUNK = N_TOK // CH    # 7
NBLK = CH // 112        # 4 two-row attention blocks per chunk
EPS = 1e-5


# ----------------------------------------------------------------------------
# Device program
# ----------------------------------------------------------------------------
def _build_program():
    import concourse.bass as bass
    import concourse.tile as tile
    from concourse import mybir

    F32 = mybir.dt.float32
    BF16 = mybir.dt.bfloat16
    AF = mybir.ActivationFunctionType
    OP = mybir.AluOpType

    nc = bass.Bass(trn_type="TRN2", target_bir_lowering=False, debug=False)
    din = {}
    for name, shape, dt_ in [
        ("x0", (128, B_LOC, T_DIM), BF16), ("x1", (32, B_LOC, T_DIM), BF16),
        ("w1t", (128, 2, N_TOK), BF16),
        ("wqgt", (128, 6, S_DIM), BF16), ("uq", (1, S_DIM), BF16),
        ("wkt", (128, 4, S_DIM), BF16),
        ("ones", (128, 128), BF16), ("ident", (128, 128), BF16),
        ("cq", (128, 4, N_TOK), F32), ("bks", (128, 4), F32),
        ("g2", (128, 4), F32), ("pe2p", (128, 4, N_TOK), F32),
        ("masks", (112, 112), F32),
        ("y", (B_LOC, 128, 4, N_TOK), F32),
    ]:
        din[name] = nc.dram_tensor(name, list(shape), dt_, kind="ExternalInput").ap()
    dout = nc.dram_tensor("out", [B_LOC, 128, 4, N_TOK], F32,
                          kind="ExternalOutput").ap()

    from contextlib import ExitStack

    with nc.allow_low_precision(reason="bf16 matmul operands, fp32 accumulate"), \
         tile.TileContext(nc) as tc, ExitStack() as ctx:
        singles = ctx.enter_context(tc.tile_pool(name="singles", bufs=1))
        io3 = ctx.enter_context(tc.tile_pool(name="io3", bufs=3))
        io2 = ctx.enter_context(tc.tile_pool(name="io2", bufs=2))
        wk2 = ctx.enter_context(tc.tile_pool(name="wk2", bufs=2))
        att = ctx.enter_context(tc.tile_pool(name="att", bufs=3))
        sc = ctx.enter_context(tc.tile_pool(name="sc", bufs=2))
        ps_mm = ctx.enter_context(tc.tile_pool(name="ps_mm", bufs=4, space="PSUM"))
        ps_att = ctx.enter_context(tc.tile_pool(name="ps_att", bufs=2, space="PSUM"))
        ps_st = ctx.enter_context(tc.tile_pool(name="ps_st", bufs=2, space="PSUM"))

        def load(name, shape, dt_):
            t = singles.tile(list(shape), dt_, tag=name)
            nc.sync.dma_start(out=t, in_=din[name])
            return t

        x0 = load("x0", (128, B_LOC, T_DIM), BF16)
        x1 = load("x1", (32, B_LOC, T_DIM), BF16)
        w1t = load("w1t", (128, 2, N_TOK), BF16)
        wqgt = load("wqgt", (128, 6, S_DIM), BF16)
        uq = load("uq", (1, S_DIM), BF16)
        wkt = load("wkt", (128, 4, S_DIM), BF16)
        ones = load("ones", (128, 128), BF16)
        ident = load("ident", (128, 128), BF16)
        bks = load("bks", (128, 4), F32)
        g2 = load("g2", (128, 4), F32)
        masks = load("masks", (112, 112), F32)
        ones_col = ones[:, 0:1]
        ones_row = ones[0:1, :]
        eps1 = singles.tile([1, 1], F32)
        nc.vector.memset(eps1, EPS)

        x_k = [(x0, 128), (x1, 21)]

        def phase_load(st):
            b, cols = st["b"], st["cols"]
            y_t = io3.tile([128, 4, CH], F32, tag="y")
            nc.sync.dma_start(out=y_t, in_=din["y"][b, :, :, cols])
            ybf = wk2.tile([128, 4, CH], BF16, tag="ybf")
            nc.scalar.activation(out=ybf, in_=y_t, func=AF.Copy)
            st["y_t"], st["ybf"] = y_t, ybf

        def phase_xe(st):
            b, cols = st["b"], st["cols"]
            xe = wk2.tile([128, 6, CH], BF16, tag="xe")
            sq = wk2.tile([128, 6, CH], BF16, tag="sq")
            for m in range(6):
                pxe = ps_mm.tile([128, CH], F32, tag="mm")
                for ik, (xt, kv) in enumerate(x_k):
                    nc.tensor.matmul(
                        pxe, xt[:kv, b, m * 128:(m + 1) * 128],
                        w1t[:kv, ik, cols], start=(ik == 0), stop=(ik == 1))
                if m % 2 == 0:
                    nc.vector.tensor_copy(out=xe[:, m, :], in_=pxe)
                else:
                    nc.scalar.activation(out=xe[:, m, :], in_=pxe, func=AF.Copy)
                nc.scalar.square(out=sq[:, m, :], in_=pxe)
            ps1 = ps_st.tile([1, CH], F32, tag="st")
            for m in range(6):
                nc.tensor.matmul(ps1, ones_col, xe[:, m, :],
                                 start=(m == 0), stop=(m == 5))
            pq1 = ps_st.tile([1, CH], F32, tag="st")
            for m in range(6):
                nc.tensor.matmul(pq1, ones_col, sq[:, m, :],
                                 start=(m == 0), stop=(m == 5))
            st["xe"], st["ps1"], st["pq1"] = xe, ps1, pq1

        def _rsqrt_row(psum_s, psum_q, inv_d):
            mrow = sc.tile([1, CH], BF16, tag="ma")
            nc.scalar.activation(out=mrow, in_=psum_s, func=AF.Copy, scale=-inv_d)
            vrow = sc.tile([1, CH], F32, tag="vb")
            nc.vector.tensor_scalar_mul(out=vrow, in0=psum_q, scalar1=inv_d)
            t = sc.tile([1, CH], F32, tag="t")
            nc.vector.tensor_mul(out=t, in0=mrow, in1=mrow)
            nc.vector.tensor_tensor(out=vrow, in0=vrow, in1=t, op=OP.subtract)
            nc.scalar.activation(out=vrow, in_=vrow, func=AF.Sqrt, bias=eps1)
            rrow = sc.tile([1, CH], BF16, tag="rr")
            nc.vector.reciprocal(out=rrow, in_=vrow)
            return mrow, rrow

        def phase_stats1(st):
            mrow1, rrow1 = _rsqrt_row(st["ps1"], st["pq1"], 1.0 / T_DIM)
            prb = ps_st.tile([128, CH], F32, tag="st")
            nc.tensor.matmul(prb, ones_row, rrow1, start=True, stop=True)
            r1b = wk2.tile([128, CH], F32, tag="r1b")
            nc.vector.tensor_copy(out=r1b, in_=prb)
            st["mrow1"], st["r1b"] = mrow1, r1b

        def phase_q(st):
            xe, mrow1, r1b = st["xe"], st["mrow1"], st["r1b"]
            cq_t = st["cq_t"]
            q = wk2.tile([128, 4, CH], BF16, tag="q")
            for oc in range(4):
                pq = ps_mm.tile([128, CH], F32, tag="mm")
                for kc in range(6):
                    nc.tensor.matmul(
                        pq, wqgt[:, kc, oc * 128:(oc + 1) * 128],
                        xe[:, kc, :], start=(kc == 0), stop=False)
                nc.tensor.matmul(pq, uq[:, oc * 128:(oc + 1) * 128], mrow1,
                                 start=False, stop=True)
                nc.vector.tensor_mul(out=q[:, oc, :], in0=pq, in1=r1b)
                nc.vector.tensor_add(out=q[:, oc, :], in0=q[:, oc, :],
                                     in1=cq_t[:, oc, :])
            st["q"] = q

        def phase_stats2a(st):
            ybf = st["ybf"]
            sq2 = wk2.tile([128, 6, CH], BF16, tag="sq")
            nc.scalar.square(out=sq2[:, 0:4, :], in_=ybf)
            ps2 = ps_st.tile([1, CH], F32, tag="st")
            for m in range(4):
                nc.tensor.matmul(ps2, ones_col, ybf[:, m, :],
                                 start=(m == 0), stop=(m == 3))
            pq2 = ps_st.tile([1, CH], F32, tag="st")
            for m in range(4):
                nc.tensor.matmul(pq2, ones_col, sq2[:, m, :],
                                 start=(m == 0), stop=(m == 3))
            st["ps2"], st["pq2"] = ps2, pq2

        def phase_stats2b(st):
            mrow2, rrow2 = _rsqrt_row(st["ps2"], st["pq2"], 1.0 / S_DIM)
            pmb2 = ps_st.tile([128, CH], F32, tag="st")
            nc.tensor.matmul(pmb2, ones_row, mrow2, start=True, stop=True)
            prb2 = ps_st.tile([128, CH], F32, tag="st")
            nc.tensor.matmul(prb2, ones_row, rrow2, start=True, stop=True)
            r2b = wk2.tile([128, CH], F32, tag="r2b")
            nc.vector.tensor_copy(out=r2b, in_=prb2)
            c2b = wk2.tile([128, CH], F32, tag="c2b")
            nc.vector.tensor_tensor(out=c2b, in0=pmb2, in1=r2b, op=OP.mult)
            st["r2b"], st["c2b"] = r2b, c2b

        def phase_ny(st):
            y_t, r2b, c2b, pe2_t = st["y_t"], st["r2b"], st["c2b"], st["pe2_t"]
            ny = wk2.tile([128, 4, CH], BF16, tag="ny")
            nyf = wk2.tile([128, 4, CH], F32, tag="nyf")
            for co in range(4):
                nc.vector.tensor_mul(out=nyf[:, co, :], in0=y_t[:, co, :], in1=r2b)
                nc.gpsimd.tensor_add(out=nyf[:, co, :], in0=nyf[:, co, :], in1=c2b)
                nc.scalar.activation(out=nyf[:, co, :], in_=nyf[:, co, :],
                                     func=AF.Identity, scale=g2[:, co:co + 1])
                nc.vector.tensor_tensor(out=ny[:, co, :], in0=nyf[:, co, :],
                                        in1=pe2_t[:, co, :], op=OP.add)
            st["ny"] = ny

        def phase_kv(st):
            ny = st["ny"]
            k = wk2.tile([128, 4, CH], BF16, tag="k")
            for oc in range(4):
                pk = ps_mm.tile([128, CH], F32, tag="mm")
                for kc in range(4):
                    nc.tensor.matmul(
                        pk, wkt[:, kc, oc * 128:(oc + 1) * 128],
                        ny[:, kc, :], start=(kc == 0), stop=(kc == 3))
                nc.vector.tensor_scalar(out=k[:, oc, :], in0=pk,
                                        scalar1=bks[:, oc:oc + 1],
                                        scalar2=None, op0=OP.add)
            v = wk2.tile([112, 4, S_DIM], BF16, tag="v")
            for blk in range(NBLK):
                tb = slice(blk * 112, (blk + 1) * 112)
                for co in range(4):
                    pt = ps_att.tile([112, 128], BF16, tag="at")
                    nc.tensor.transpose(pt, ny[:, co, tb], ident)
                    if co % 2 == 0:
                        nc.vector.tensor_copy(
                            out=v[:, blk, co * 128:(co + 1) * 128], in_=pt)
                    else:
                        nc.scalar.activation(
                            out=v[:, blk, co * 128:(co + 1) * 128], in_=pt,
                            func=AF.Copy)
            st["k"], st["v"] = k, v

        def phase_att(st):
            b, cols = st["b"], st["cols"]
            q, k, v, y_t = st["q"], st["k"], st["v"], st["y_t"]
            out_t = io2.tile([128, 4, CH], F32, tag="out")
            for blk in range(NBLK):
                tb = slice(blk * 112, (blk + 1) * 112)
                psc = ps_att.tile([112, 112], F32, tag="at")
                for oc in range(4):
                    nc.tensor.matmul(psc, q[:, oc, tb], k[:, oc, tb],
                                     start=(oc == 0), stop=(oc == 3))
                e_t = att.tile([112, 112], F32, tag="e")
                nc.vector.tensor_add(out=e_t, in0=psc, in1=masks)
                den = att.tile([112, 1], F32, tag="den")
                nc.scalar.activation(out=e_t, in_=e_t, func=AF.Exp, accum_out=den)
                nc.vector.reciprocal(out=den, in_=den)
                attn = att.tile([112, 112], BF16, tag="attn")
                nc.vector.tensor_scalar_mul(out=attn, in0=e_t, scalar1=den)
                pat = ps_att.tile([112, 112], BF16, tag="at")
                nc.tensor.transpose(pat, attn, ident[:112, :112])
                attnT = att.tile([112, 112], BF16, tag="attnT")
                nc.vector.tensor_copy(out=attnT, in_=pat)
                pav = ps_mm.tile([128, 4, 112], F32, tag="mm")
                for co in range(4):
                    nc.tensor.matmul(pav[:, co, :],
                                     v[:, blk, co * 128:(co + 1) * 128],
                                     attnT, start=True, stop=True)
                nc.vector.tensor_add(out=out_t[:, :, tb], in0=pav,
                                     in1=y_t[:, :, tb])
            nc.sync.dma_start(out=dout[b, :, :, cols], in_=out_t)

        for ich in range(NCHUNK):
            cols = slice(ich * CH, (ich + 1) * CH)
            cq_t = io2.tile([128, 4, CH], F32, tag="cq")
            nc.sync.dma_start(out=cq_t, in_=din["cq"][:, :, cols])
            pe2_t = io2.tile([128, 4, CH], F32, tag="pe2")
            nc.sync.dma_start(out=pe2_t, in_=din["pe2p"][:, :, cols])

            s0 = {"b": 0, "cols": cols, "cq_t": cq_t, "pe2_t": pe2_t}
            s1 = {"b": 1, "cols": cols, "cq_t": cq_t, "pe2_t": pe2_t}

            phase_load(s0)
            phase_xe(s0)
            phase_load(s1)
            phase_stats1(s0)
            phase_stats2a(s0)
            phase_xe(s1)
            phase_stats2b(s0)
            phase_q(s0)
            phase_stats1(s1)
            phase_ny(s0)
            phase_stats2a(s1)
            phase_stats2b(s1)
            phase_q(s1)
            phase_kv(s0)
            phase_ny(s1)
            phase_att(s0)
            phase_kv(s1)
            phase_att(s1)
    return nc


# ----------------------------------------------------------------------------
# Host-side preparation
# ----------------------------------------------------------------------------
def _make_const_inputs(W_conv1, b_conv1, ln1_g, ln1_b, ln2_g, ln2_b,
                       pe_wave, pe_spec, Wq, bq, Wk, bk):
    import ml_dtypes
    f = np.float32
    bf = ml_dtypes.bfloat16
    s = np.float32(S_DIM) ** np.float32(-0.25)

    w1t = np.zeros((128, 2, N_TOK), dtype=f)
    w1T = W_conv1.T.astype(f)
    w1t[:, 0, :] = w1T[:128]
    w1t[:21, 1, :] = w1T[128:]

    wqg = (Wq * ln1_g[None, :]).astype(f) * s
    wqgt = wqg.T.reshape(6, 128, S_DIM).transpose(1, 0, 2).copy()
    uq = (Wq @ ln1_g).astype(f)[None, :] * s

    pe_w = pe_wave.reshape(T_DIM, N_TOK).astype(f)
    cq = (Wq @ (ln1_b[:, None] + pe_w)).astype(f) * s + (bq[:, None] * s).astype(f)
    cq = cq.reshape(4, 128, N_TOK).transpose(1, 0, 2).copy()

    wkt = (Wk.T * s).astype(f).reshape(4, 128, S_DIM).transpose(1, 0, 2).copy()
    bks = (bk * s).astype(f).reshape(4, 128).T.copy()
    g2 = ln2_g.astype(f).reshape(4, 128).T.copy()

    pe2p = (pe_spec.reshape(S_DIM, N_TOK) + ln2_b[:, None]).astype(f)
    pe2p = pe2p.reshape(4, 128, N_TOK).transpose(1, 0, 2).copy()

    masks = np.full((112, 112), -1e30, dtype=f)
    for sb in range(2):
        masks[sb * 56:(sb + 1) * 56, sb * 56:(sb + 1) * 56] = 0.0

    return {
        "w1t": w1t.astype(bf), "wqgt": wqgt.astype(bf), "uq": uq.astype(bf),
        "cq": cq, "wkt": wkt.astype(bf), "bks": bks, "g2": g2,
        "pe2p": pe2p, "masks": masks,
        "ones": np.ones((128, 128), dtype=bf),
        "ident": np.eye(128, dtype=bf),
    }


def _make_core_inputs(consts, x_shard, y_shard):
    import ml_dtypes
    f = np.float32
    bf = ml_dtypes.bfloat16
    x0 = x_shard[:, :128, :].transpose(1, 0, 2).astype(bf).copy()
    x1 = np.zeros((32, B_LOC, T_DIM), dtype=bf)
    x1[:21] = x_shard[:, 128:, :].transpose(1, 0, 2).astype(bf)
    y = y_shard.reshape(B_LOC, 4, 128, N_TOK).transpose(0, 2, 1, 3).astype(f).copy()
    m = {"x0": x0, "x1": x1, "y": y}
    m.update(consts)
    return m


_cached_nc = [None]


def kernel(x, y, W_conv1, b_conv1, ln1_g, ln1_b, ln2_g, ln2_b,
           pe_wave, pe_spec, Wq, bq, Wk, bk):
    _install_patch()
    from concourse.bass_utils import run_bass_kernel_spmd

    x = np.asarray(x, dtype=np.float32)
    y = np.asarray(y, dtype=np.float32)
    consts = _make_const_inputs(
        np.asarray(W_conv1, np.float32), np.asarray(b_conv1, np.float32),
        np.asarray(ln1_g, np.float32), np.asarray(ln1_b, np.float32),
        np.asarray(ln2_g, np.float32), np.asarray(ln2_b, np.float32),
        np.asarray(pe_wave, np.float32), np.asarray(pe_spec, np.float32),
        np.asarray(Wq, np.float32), np.asarray(bq, np.float32),
        np.asarray(Wk, np.float32), np.asarray(bk, np.float32))
    in_maps = [
        _make_core_inputs(consts, x[B_LOC * i:B_LOC * (i + 1)],
                          y[B_LOC * i:B_LOC * (i + 1)])
        for i in range(N_CORES)
    ]

    if _cached_nc[0] is None:
        _cached_nc[0] = _build_program()
    nc = _cached_nc[0]

    res = run_bass_kernel_spmd(nc, in_maps, core_ids=list(range(N_CORES)))
    outs = []
    for i in range(N_CORES):
        o = res.results[i]["out"]  # (B_LOC, 128, 4, N_TOK)
        outs.append(o.transpose(0, 2, 1, 3).reshape(B_LOC, S_DIM, H, W))
    return np.concatenate(outs, axis=0).astype(np.float32)
